# revision 2
# baseline (speedup 1.0000x reference)
"""DarkChannel Trainium2 kernel (fp16 pipeline).

Computes, per image: channel-min over C=3, then 15x15 sliding-window min
with reflect padding (== clamped-window min) over [B,3,512,512] f32
-> [B,1,512,512] f32.

Sharding: pure data parallel, batch 16 -> 2 images on each of 8 cores.

Key calibration facts for this silicon (measured via differential timing):
  - DVE ~1.2 GHz; fp32 TT = 1 elem/cycle/lane; fp16 TT = 4x when both
    operands 4B-aligned, 2x when an operand is 2B-misaligned; TT with
    f32 inputs and f16 output runs ~2x; mixed f16+f32 -> f16 runs ~2x.
  - PE transpose 128x128 ~0.3us/instr; xbar DMA transpose ~0.6us/block
    (slower than PE).  Partition-offset TT operands and GpSimd TT are
    rejected by this walrus build.
  - HBM ~358 GB/s/core -> load 6.29MB + store 1.05MB(f16) ~ 20.5us floor.

Pipeline per image (per core):
  1. DMA 3 f32 channel planes into SBUF, rows on partitions.
  2. chan-min: T(f16) = min(c0,c1) [f32->f16 TT], padded Pb(f16) =
     min(T, c2) [mixed TT].  Pad cols = 2.0 (> any input; inputs < 1).
  3. Horizontal 15-tap min: log-shifts (1,2,4,7) in fp16; shifts 2,4 hit
     the 4x DVE mode, shifts 1,7 the 2x mode.
  4. Transpose via TensorE (f16 PSUM), ScalarE copies into padded
     vertical buffers; vertical chain same as 3; transpose back.
  5. DMA out in f16; host upcasts to f32 (max rel err ~2^-11 from the
     single f32->f16 rounding; min itself is exact in f16).
"""

import numpy as np

import concourse.bacc as bacc
import concourse.mybir as mybir
from concourse.tile import TileContext
from concourse.masks import make_identity
from concourse.bass_utils import run_bass_kernel_spmd

F32 = mybir.dt.float32
F16 = mybir.dt.float16
MIN = mybir.AluOpType.min

P = 128          # SBUF partitions
H = W = 512
NT = 4           # row-tiles (128 rows each) per image
PAD = 7
PW = W + 2 * PAD  # 526
BIG = 2.0        # > max input value (inputs in [0,1)); f16-safe
B_PER_CORE = 2
N_CORES = 8


def _build(repeat=1, n_images=B_PER_CORE, ngrp=2, vgrp=1, hsplit=1, vsplit=1,
           split_load=True, split_store=True, xin_bufs=2,
           work_bufs=4, himg_bufs=2, out_bufs=2, psum_bufs=4):
    """Build + compile the Bacc program. Returns nc.

    ngrp: row-groups for the chan-min stage (finer = earlier start after
    partial loads). vgrp: groups for the vertical buffers. hsplit/vsplit:
    instruction splitting of the shift chains (1 = whole-image 3D APs)."""
    tpg = NT // ngrp
    tpv = NT // vgrp
    nc = bacc.Bacc("TRN2", target_bir_lowering=False, debug=False)
    x = nc.declare_dram_parameter("x", [n_images, 3, H, W], F32, isOutput=False)
    y = nc.declare_dram_parameter("y", [n_images, 1, H, W], F16, isOutput=True)

    with TileContext(nc) as tc:
        with (
            tc.tile_pool(name="consts", bufs=1) as consts,
            tc.tile_pool(name="xin", bufs=xin_bufs) as xin_pool,
            tc.tile_pool(name="tmp", bufs=2) as tmp_pool,
            tc.tile_pool(name="work", bufs=work_bufs) as work_pool,
            tc.tile_pool(name="himg", bufs=himg_bufs) as h_pool,
            tc.tile_pool(name="outp", bufs=out_bufs) as out_pool,
            tc.tile_pool(name="ps", bufs=psum_bufs, space="PSUM") as psum_pool,
        ):
            ident = consts.tile([P, P], F16)
            make_identity(nc, ident)

            def _chain(buf, nt, split, out_pool_, out_tag):
                """fp16 log-shift min chain over the innermost axis."""
                cur, wid = buf, PW
                for s in (1, 2, 4, 7):
                    nw = wid - s
                    if s != 7:
                        nxt = work_pool.tile([P, nt, PW], F16, tag="work")
                        ow = nw
                    else:
                        nxt = out_pool_.tile([P, nt, W], F16, tag=out_tag)
                        ow = W
                    if split == 1:
                        nc.vector.tensor_tensor(
                            out=nxt[:, :, 0:nw], in0=cur[:, :, 0:nw],
                            in1=cur[:, :, s:s + nw], op=MIN,
                        )
                    else:
                        tps = nt // split
                        for g in range(split):
                            t0, t1 = g * tps, (g + 1) * tps
                            nc.vector.tensor_tensor(
                                out=nxt[:, t0:t1, 0:nw], in0=cur[:, t0:t1, 0:nw],
                                in1=cur[:, t0:t1, s:s + nw], op=MIN,
                            )
                    cur, wid = nxt, nw
                return cur

            for _rep in range(repeat):
                for b in range(n_images):
                    # ---- load: 3 channel planes, rows->partitions ----
                    X = xin_pool.tile([P, 3, NT, W], F32, tag="xin")
                    xr = x[b].rearrange("c (i p) w -> p c i w", p=P)
                    if split_load:
                        for hlf in range(2):
                            i0, i1 = hlf * 2, hlf * 2 + 2
                            for c in range(3):
                                nc.sync.dma_start(
                                    out=X[:, c, i0:i1], in_=xr[:, c, i0:i1]
                                )
                    else:
                        for c in range(3):
                            nc.sync.dma_start(out=X[:, c], in_=xr[:, c])

                    # ---- chan-min into ONE padded fp16 buffer ----
                    Pb = work_pool.tile([P, NT, PW], F16, tag="work")
                    nc.vector.memset(Pb[:, :, 0:PAD], BIG)
                    nc.vector.memset(Pb[:, :, PAD + W:PW], BIG)
                    for g in range(ngrp):
                        t0, t1 = g * tpg, (g + 1) * tpg
                        T = tmp_pool.tile([P, tpg, W], F16, tag="tmp")
                        nc.vector.tensor_tensor(
                            out=T[:], in0=X[:, 0, t0:t1],
                            in1=X[:, 1, t0:t1], op=MIN,
                        )
                        nc.vector.tensor_tensor(
                            out=Pb[:, t0:t1, PAD:PAD + W], in0=T[:],
                            in1=X[:, 2, t0:t1], op=MIN,
                        )

                    # ---- horizontal erosion ----
                    hmin = _chain(Pb, NT, hsplit, h_pool, "himg")

                    # ---- transpose -> padded vertical buffers ----
                    vbufs = []
                    for g in range(vgrp):
                        Vb = work_pool.tile([P, tpv, PW], F16, tag="work")
                        nc.vector.memset(Vb[:, :, 0:PAD], BIG)
                        nc.vector.memset(Vb[:, :, PAD + W:PW], BIG)
                        for jj in range(tpv):
                            j = g * tpv + jj  # absolute col-tile
                            TP = psum_pool.tile([P, W], F16, tag="tp")
                            for i in range(NT):
                                nc.tensor.transpose(
                                    TP[:, i * P:(i + 1) * P],
                                    hmin[:, i, j * P:(j + 1) * P],
                                    ident,
                                )
                            nc.scalar.copy(out=Vb[:, jj, PAD:PAD + W], in_=TP[:])
                        vbufs.append(Vb)

                    # ---- vertical erosion (free axis = rows now) ----
                    vmins = []
                    for g in range(vgrp):
                        vmins.append(
                            _chain(vbufs[g], tpv, vsplit, h_pool, "vimg"))

                    # ---- transpose back + store ----
                    yr = y[b, 0].rearrange("(i p) w -> p i w", p=P)
                    OUT = out_pool.tile([P, NT, W], F16, tag="outp")
                    for i in range(NT):
                        TO = psum_pool.tile([P, W], F16, tag="to")
                        for j in range(NT):
                            vg = vmins[j // tpv]
                            nc.tensor.transpose(
                                TO[:, j * P:(j + 1) * P],
                                vg[:, j % tpv, i * P:(i + 1) * P],
                                ident,
                            )
                        nc.scalar.copy(out=OUT[:, i], in_=TO[:])
                        if split_store:
                            nc.sync.dma_start(out=yr[:, i], in_=OUT[:, i])
                    if not split_store:
                        nc.sync.dma_start(out=yr, in_=OUT[:])
    nc.compile()
    return nc


_CACHE = {}


def _get_nc(**kw):
    key = tuple(sorted(kw.items()))
    if key not in _CACHE:
        _CACHE[key] = _build(**kw)
    return _CACHE[key]


def kernel(x: np.ndarray) -> np.ndarray:
    """Full-input entry point: x [16,3,512,512] f32 -> [16,1,512,512] f32."""
    x = np.ascontiguousarray(x, dtype=np.float32)
    B = x.shape[0]
    assert B == N_CORES * B_PER_CORE, x.shape
    nc = _get_nc()
    in_maps = [
        {"x": x[c * B_PER_CORE:(c + 1) * B_PER_CORE]} for c in range(N_CORES)
    ]
    res = run_bass_kernel_spmd(nc, in_maps, core_ids=list(range(N_CORES)))
    out = np.concatenate([res.results[c]["y"] for c in range(N_CORES)], axis=0)
    return out.astype(np.float32)
